# revision 4
# baseline (speedup 1.0000x reference)
"""Trainium2 Bass kernel for nn_BDH_69638599737422 (dense_transformer).

Sharding (8 NeuronCores): core c = 2*h + j owns head h (of 4) and N-half j
(4096 of 8192 latent dims). encoder/encoder_v column-parallel, decoder
row-parallel. Per layer: one 2-rank AllReduce (partial yKV within a head
pair, since scores contract over the full head N) and one 8-rank AllReduce
(y = xy @ decoder partial sums into D).

All on-device tensors are fp16 (PE matmuls run fp16 at full rate with fp32
PSUM accumulation; verified ~1.3e-3 rel err vs the fp32 reference).

The RoPE frequency table repeats in pairs (quantize(t,2)), so a host-side
even/odd de-interleave permutation of each core's N slice (baked into
encoder/encoder_v columns and decoder rows) turns rotate_half into two
contiguous halves: qe = xe*c - xo*s, qo = xo*c + xe*s.

The causal mask (strict lower) is applied on the transposed score matrix
P[s,t] = scores[t,s]: Q@Q^T is symmetric, so P comes out of the same
matmuls and the mask becomes strict-upper, which lets the per-s-chunk
matmuls skip the all-zero left region entirely (triangle skip).
"""

import numpy as np

import concourse.bass as bass
import concourse.tile as tile
from concourse import bacc, mybir
from concourse.bass_utils import run_bass_kernel_spmd
from concourse.masks import make_identity

F16 = mybir.dt.float16
F32 = mybir.dt.float32
AF = mybir.ActivationFunctionType
ALU = mybir.AluOpType

B, T, D, NH, VOCAB = 1, 512, 256, 4, 256
N = 8192        # latent dim per head
NL = 4096       # per-core N slice
NPAIR = 2048    # rope pairs per core
NT = NL // 128  # 32 n-tiles per core
N_LAYER = 6
EPS = 1e-5
THETA = 2.0 ** 16
TWO_PI = 2.0 * np.pi
CORES = list(range(8))
PAIR_GROUPS = [[0, 1], [2, 3], [4, 5], [6, 7]]

_STATE = {}


# ---------------------------------------------------------------- host prep

def _ln_np(x):
    m = x.mean(-1, keepdims=True)
    v = ((x - m) ** 2).mean(-1, keepdims=True)
    return (x - m) / np.sqrt(v + EPS)


def _rope_pair_tables():
    """cos/sin at even lanes only (freqs repeat in pairs): [T, N//2] f32."""
    t = np.arange(N, dtype=np.float32)
    q = (np.floor(t / 2.0) * 2.0).astype(np.float32)
    freqs = (1.0 / (THETA ** (q / np.float32(N))) / np.float32(TWO_PI)).astype(
        np.float32
    )
    pos = np.arange(T, dtype=np.float32)
    ang = ((pos[:, None] * freqs[None, :]) % 1.0) * np.float32(TWO_PI)
    cos = np.cos(ang).astype(np.float32)
    sin = np.sin(ang).astype(np.float32)
    return cos[:, ::2], sin[:, ::2]


def _tileize_rows(a, rows_per_tile=128):
    """[n_tiles*128, w] -> [128, n_tiles*w] with free dim = (tile, w)."""
    r, w = a.shape
    nt = r // rows_per_tile
    return np.ascontiguousarray(
        a.reshape(nt, rows_per_tile, w).transpose(1, 0, 2).reshape(rows_per_tile, nt * w)
    )


def _build_in_maps(idx, embed, encoder, encoder_v, decoder, lm_head):
    idx = np.asarray(idx)
    embed = np.asarray(embed, dtype=np.float32)
    encoder = np.asarray(encoder, dtype=np.float32)
    encoder_v = np.asarray(encoder_v, dtype=np.float32)
    decoder = np.asarray(decoder, dtype=np.float32)
    lm_head = np.asarray(lm_head, dtype=np.float32)

    x0 = _ln_np(embed[idx[0]]).astype(np.float16)          # [T, D]
    x_td0 = _tileize_rows(x0)                               # [128, 4*256]
    x_dt0 = _tileize_rows(np.ascontiguousarray(x0.T))       # [128, 2*512]

    cos_p, sin_p = _rope_pair_tables()                      # [T, 4096] f32
    # even lanes first, then odd lanes
    perm = np.concatenate([np.arange(0, NL, 2), np.arange(1, NL, 2)])

    maskd = np.triu(np.ones((128, 128), np.float16), k=1)   # keep s < t
    lmh = _tileize_rows(lm_head.astype(np.float16))         # [128, 2*256]

    in_maps = []
    for c in CORES:
        h, j = c // 2, c % 2
        nsl = slice(j * NL, (j + 1) * NL)
        enc_s = encoder[h][:, nsl][:, perm].astype(np.float16)      # [256, 4096]
        ev_s = encoder_v[h][:, nsl][:, perm].astype(np.float16)
        dec_s = decoder[h * N + j * NL : h * N + (j + 1) * NL][perm].astype(
            np.float16
        )                                                            # [4096, 256]
        kp = slice(j * NPAIR, (j + 1) * NPAIR)
        cos_s = np.ascontiguousarray(cos_p[:, kp].T).astype(np.float16)  # [2048, 512]
        sin_s = np.ascontiguousarray(sin_p[:, kp].T).astype(np.float16)
        in_maps.append(
            {
                "enc0": np.ascontiguousarray(enc_s[:128]),
                "enc1": np.ascontiguousarray(enc_s[128:]),
                "ev0": np.ascontiguousarray(ev_s[:128]),
                "ev1": np.ascontiguousarray(ev_s[128:]),
                "decb": _tileize_rows(dec_s),               # [128, 32*256]
                "cosb": _tileize_rows(cos_s),               # [128, 16*512]
                "sinb": _tileize_rows(sin_s),
                "maskd": maskd,
                "x_td0": x_td0,
                "x_dt0": x_dt0,
                "lmh": lmh,
            }
        )
    return in_maps


# ---------------------------------------------------------------- device code

def _layer_norm_chunks(nc, st, out_f16, in_ap, n_chunks, chunk, epst):
    """LN over the free dim in `chunk`-sized pieces: out = (in - mu) * rstd."""
    for tc in range(n_chunks):
        sl = slice(tc * chunk, (tc + 1) * chunk)
        stats = st.tile([128, 6], F32, tag="st6")
        mv = st.tile([128, 2], F32, tag="st2")
        nc.vector.bn_stats(out=stats, in_=in_ap[:, sl])
        nc.vector.bn_aggr(out=mv, in_=stats)
        nc.scalar.activation(
            out=mv[:, 1:2], in_=mv[:, 1:2], func=AF.Sqrt, bias=epst, scale=1.0
        )
        nc.vector.reciprocal(out=mv[:, 1:2], in_=mv[:, 1:2])
        nc.vector.tensor_scalar(
            out=out_f16[:, sl],
            in0=in_ap[:, sl],
            scalar1=mv[:, 0:1],
            scalar2=mv[:, 1:2],
            op0=ALU.subtract,
            op1=ALU.mult,
        )


def _transpose_blocks(nc, ps, dst, src, n_tc, n_dc, ident):
    """dst[(dc,t-block)] = src[(tc,d-block)]^T for [128,128] blocks.

    src free = (tc, n_dc*128), dst free = (dc, n_tc*128)."""
    for tc in range(n_tc):
        for dc in range(n_dc):
            tr = ps.tile([128, 128], F16, tag="ps")
            nc.tensor.transpose(
                tr, src[:, tc * (n_dc * 128) + dc * 128 :][:, :128], ident
            )
            nc.scalar.copy(
                out=dst[:, dc * (n_tc * 128) + tc * 128 :][:, :128], in_=tr
            )


def _build_bass():
    nc = bacc.Bacc(None, target_bir_lowering=False, num_devices=len(CORES))

    dp = nc.declare_dram_parameter
    enc0_e = dp("enc0", [128, NL], F16, isOutput=False)
    enc1_e = dp("enc1", [128, NL], F16, isOutput=False)
    ev0_e = dp("ev0", [128, NL], F16, isOutput=False)
    ev1_e = dp("ev1", [128, NL], F16, isOutput=False)
    dec_e = dp("decb", [128, NT * D], F16, isOutput=False)
    cos_e = dp("cosb", [128, 16 * T], F16, isOutput=False)
    sin_e = dp("sinb", [128, 16 * T], F16, isOutput=False)
    mask_e = dp("maskd", [128, 128], F16, isOutput=False)
    xtd_e = dp("x_td0", [128, 4 * D], F16, isOutput=False)
    xdt_e = dp("x_dt0", [128, 2 * T], F16, isOutput=False)
    lmh_e = dp("lmh", [128, 2 * VOCAB], F16, isOutput=False)
    out_e = dp("logits", [T, VOCAB], F32, isOutput=True)

    with tile.TileContext(nc) as tc_:
        pools = [
            tc_.tile_pool(name="wt", bufs=1),
            tc_.tile_pool(name="big", bufs=1),
            tc_.tile_pool(name="xp", bufs=2),
            tc_.tile_pool(name="tmp", bufs=1),
            tc_.tile_pool(name="ys", bufs=3),
            tc_.tile_pool(name="st", bufs=8),
            tc_.tile_pool(name="stg", bufs=1),
            tc_.tile_pool(name="ps", bufs=8, space="PSUM"),
            tc_.tile_pool(name="dram", bufs=2, space="DRAM"),
        ]
        wt, big, xp, tmp, ysp, st, stg, ps, dram = [p.__enter__() for p in pools]
        try:
            _emit(nc, wt, big, xp, tmp, ysp, st, stg, ps, dram,
                  enc0_e, enc1_e, ev0_e, ev1_e, dec_e, cos_e, sin_e, mask_e,
                  xtd_e, xdt_e, lmh_e, out_e)
        finally:
            for p in reversed(pools):
                p.__exit__(None, None, None)
    nc.compile()
    return nc


def _emit(nc, wt, big, xp, tmp, ysp, st, stg, ps, dram,
          enc0_e, enc1_e, ev0_e, ev1_e, dec_e, cos_e, sin_e, mask_e,
          xtd_e, xdt_e, lmh_e, out_e):
    dma = nc.sync.dma_start

    # persistent weights / tables
    enc0 = wt.tile([128, NL], F16, tag="enc0")
    enc1 = wt.tile([128, NL], F16, tag="enc1")
    ev0 = wt.tile([128, NL], F16, tag="ev0")
    ev1 = wt.tile([128, NL], F16, tag="ev1")
    dect = wt.tile([128, NT * D], F16, tag="dect")
    cost = wt.tile([128, 16 * T], F16, tag="cost")
    sint = wt.tile([128, 16 * T], F16, tag="sint")
    maskt = wt.tile([128, 128], F16, tag="maskt")
    lmht = wt.tile([128, 2 * VOCAB], F16, tag="lmht")
    ident = wt.tile([128, 128], F16, tag="ident")
    epst = wt.tile([128, 1], F32, tag="epst")

    xsb = big.tile([128, NT * T], F16, tag="xsb")    # xs then xy, (i, t)
    qrb = big.tile([128, NT * T], F16, tag="qrb")    # roped qs, (i, t)
    Pb = big.tile([128, 4 * T], F16, tag="Pb")       # masked scores^T, (sc, t)

    x_first = xp.tile([128, 4 * D], F16, tag="x_td")
    xd_first = xp.tile([128, 2 * T], F16, tag="x_dt")
    dma(out=x_first, in_=xtd_e[:])
    dma(out=xd_first, in_=xdt_e[:])
    dma(out=enc0, in_=enc0_e[:])
    dma(out=enc1, in_=enc1_e[:])
    dma(out=cost, in_=cos_e[:])
    dma(out=sint, in_=sin_e[:])
    dma(out=maskt, in_=mask_e[:])
    dma(out=ev0, in_=ev0_e[:])
    dma(out=ev1, in_=ev1_e[:])
    dma(out=dect, in_=dec_e[:])
    dma(out=lmht, in_=lmh_e[:])
    nc.vector.memset(epst, EPS)
    make_identity(nc, ident[:])

    x_td, x_dt = x_first, xd_first
    for _layer in range(N_LAYER):
        # -- phase 1: xs = relu(x @ enc), out [nl, t] tiles
        for i in range(NT):
            mm = ps.tile([128, T], F32, tag="ps")
            nc.tensor.matmul(
                out=mm, lhsT=enc0[:, i * 128 : (i + 1) * 128],
                rhs=x_dt[:, 0:T], start=True, stop=False,
            )
            nc.tensor.matmul(
                out=mm, lhsT=enc1[:, i * 128 : (i + 1) * 128],
                rhs=x_dt[:, T : 2 * T], start=False, stop=True,
            )
            nc.scalar.activation(
                out=xsb[:, i * T : (i + 1) * T], in_=mm, func=AF.Relu
            )

        # -- phase 2: rope, 4 chunks of 4 tiles
        CH = 4 * T  # 2048 columns per chunk
        for c in range(4):
            e_sl = slice(c * CH, (c + 1) * CH)
            o_sl = slice(16 * T + c * CH, 16 * T + (c + 1) * CH)
            tme = tmp.tile([128, CH], F16, tag="tmpe")
            nc.gpsimd.tensor_mul(tme, xsb[:, o_sl], sint[:, e_sl])
            nc.vector.tensor_mul(qrb[:, e_sl], xsb[:, e_sl], cost[:, e_sl])
            nc.vector.tensor_sub(qrb[:, e_sl], qrb[:, e_sl], tme)
            tmo = tmp.tile([128, CH], F16, tag="tmpo")
            nc.gpsimd.tensor_mul(tmo, xsb[:, e_sl], sint[:, e_sl])
            nc.vector.tensor_mul(qrb[:, o_sl], xsb[:, o_sl], cost[:, e_sl])
            nc.vector.tensor_add(qrb[:, o_sl], qrb[:, o_sl], tmo)

        # -- phase 3: P[s,t] = (qr^T qr) masked to s < t (triangle skip)
        P_ps = [ps.tile([128, T], F32, tag="ps", name=f"P_ps{m}") for m in range(4)]
        for jt in range(NT):
            base = jt * T
            for m in range(4):
                t0 = m * 128
                nc.tensor.matmul(
                    out=P_ps[m][:, t0:T],
                    lhsT=qrb[:, base + t0 : base + t0 + 128],
                    rhs=qrb[:, base + t0 : base + T],
                    start=(jt == 0), stop=(jt == NT - 1),
                    skip_group_check=True,
                )
        for m in range(4):
            t0 = m * 128
            if m > 0:
                nc.vector.memset(Pb[:, m * T : m * T + t0], 0.0)
            nc.vector.tensor_mul(
                Pb[:, m * T + t0 : m * T + t0 + 128],
                P_ps[m][:, t0 : t0 + 128],
                maskt,
            )
            if m < 3:
                nc.scalar.copy(
                    out=Pb[:, m * T + t0 + 128 : (m + 1) * T],
                    in_=P_ps[m][:, t0 + 128 : T],
                )

        # -- phase 4: yKV partial = P^T-contraction with x (V), pair AllReduce, LN
        ykv_ps = [ps.tile([128, D], F32, tag="ps", name=f"ykv_ps{m}") for m in range(4)]
        for tcn in range(4):
            for sc in range(4):
                nc.tensor.matmul(
                    out=ykv_ps[tcn],
                    lhsT=Pb[:, sc * T + tcn * 128 : sc * T + (tcn + 1) * 128],
                    rhs=x_td[:, sc * D : (sc + 1) * D],
                    start=(sc == 0), stop=(sc == 3),
                    skip_group_check=True,
                )
        stage1 = stg.tile([128, 4 * D], F32, tag="stg1")
        for tcn in range(4):
            nc.vector.tensor_copy(
                out=stage1[:, tcn * D : (tcn + 1) * D], in_=ykv_ps[tcn]
            )
        b1i = dram.tile([128, 4 * D], F32, tag="b1i")
        b1o = dram.tile([128, 4 * D], F32, tag="b1o")
        dma(out=b1i, in_=stage1)
        nc.gpsimd.collective_compute(
            "AllReduce", ALU.add, replica_groups=PAIR_GROUPS,
            ins=[b1i.opt()], outs=[b1o.opt()],
        )
        ykvsum = stg.tile([128, 4 * D], F32, tag="ykvsum")
        dma(out=ykvsum, in_=b1o)

        ykv_td = xp.tile([128, 4 * D], F16, tag="ykv_td")
        _layer_norm_chunks(nc, st, ykv_td, ykvsum, 4, D, epst)
        ykv_dt = xp.tile([128, 2 * T], F16, tag="ykv_dt")
        _transpose_blocks(nc, ps, ykv_dt, ykv_td, 4, 2, ident)

        # -- phase 5: ys = relu(yKV @ encv); xy = xs*ys; y += xy @ dec
        y_ps = [ps.tile([128, D], F32, tag="ps", name=f"y_ps{m}") for m in range(4)]
        for i in range(NT):
            mm = ps.tile([128, T], F32, tag="ps")
            nc.tensor.matmul(
                out=mm, lhsT=ev0[:, i * 128 : (i + 1) * 128],
                rhs=ykv_dt[:, 0:T], start=True, stop=False,
            )
            nc.tensor.matmul(
                out=mm, lhsT=ev1[:, i * 128 : (i + 1) * 128],
                rhs=ykv_dt[:, T : 2 * T], start=False, stop=True,
            )
            ys_sb = ysp.tile([128, T], F16, tag="ys")
            nc.scalar.activation(out=ys_sb, in_=mm, func=AF.Relu)
            nc.vector.tensor_mul(
                xsb[:, i * T : (i + 1) * T], xsb[:, i * T : (i + 1) * T], ys_sb
            )
            for tcn in range(4):
                nc.tensor.matmul(
                    out=y_ps[tcn],
                    lhsT=xsb[:, i * T + tcn * 128 : i * T + (tcn + 1) * 128],
                    rhs=dect[:, i * D : (i + 1) * D],
                    start=(i == 0), stop=(i == NT - 1),
                    skip_group_check=True,
                )

        # -- phase 6: 8-core AllReduce of y partials
        stage2 = stg.tile([128, 4 * D], F32, tag="stg2")
        for tcn in range(4):
            nc.vector.tensor_copy(
                out=stage2[:, tcn * D : (tcn + 1) * D], in_=y_ps[tcn]
            )
        b2i = dram.tile([128, 4 * D], F32, tag="b2i")
        b2o = dram.tile([128, 4 * D], F32, tag="b2o")
        dma(out=b2i, in_=stage2)
        nc.gpsimd.collective_compute(
            "AllReduce", ALU.add, replica_groups=[CORES],
            ins=[b2i.opt()], outs=[b2o.opt()],
        )
        ysum = stg.tile([128, 4 * D], F32, tag="ysum")
        dma(out=ysum, in_=b2o)

        # -- phase 7: y = LN(ysum); x = LN(x + y); refresh x_dt
        y_ln = xp.tile([128, 4 * D], F16, tag="y_ln")
        _layer_norm_chunks(nc, st, y_ln, ysum, 4, D, epst)
        z = xp.tile([128, 4 * D], F16, tag="z")
        nc.vector.tensor_add(z, x_td, y_ln)
        x_td = xp.tile([128, 4 * D], F16, tag="x_td")
        _layer_norm_chunks(nc, st, x_td, z, 4, D, epst)
        x_dt = xp.tile([128, 2 * T], F16, tag="x_dt")
        _transpose_blocks(nc, ps, x_dt, x_td, 4, 2, ident)

    # -- logits = x @ lm_head
    for tcn in range(4):
        lg = ps.tile([128, VOCAB], F32, tag="ps")
        for dc in range(2):
            nc.tensor.matmul(
                out=lg,
                lhsT=x_dt[:, dc * T + tcn * 128 : dc * T + (tcn + 1) * 128],
                rhs=lmht[:, dc * VOCAB : (dc + 1) * VOCAB],
                start=(dc == 0), stop=(dc == 1),
            )
        lg_sb = ysp.tile([128, VOCAB], F32, tag="lg")
        nc.vector.tensor_copy(out=lg_sb, in_=lg)
        dma(out=out_e[tcn * 128 : (tcn + 1) * 128, :], in_=lg_sb)


# ---------------------------------------------------------------- entry point

def kernel(idx, embed, encoder, encoder_v, decoder, lm_head):
    if "nc" not in _STATE:
        _STATE["nc"] = _build_bass()
    nc = _STATE["nc"]
    in_maps = _build_in_maps(idx, embed, encoder, encoder_v, decoder, lm_head)
    import os

    trace = bool(int(os.environ.get("KERNEL_TRACE", "0")))
    res = run_bass_kernel_spmd(nc, in_maps, core_ids=CORES, trace=trace)
    _STATE["last_results"] = res
    return res.results[0]["logits"].reshape(B, T, VOCAB).astype(np.float32)
